# revision 1
# baseline (speedup 1.0000x reference)
"""Trainium2 Bass kernel for nn_MiddleFusionModule.

out = LayerNorm(node + sigmoid(node@Wg1 + (t@Wg2+bg)[seg]) * t[seg]),
t = relu(text@W1+b1)@W2+b2, over 131072 nodes sharded across 8 cores.

Strategy (one SPMD program, 8 data-parallel cores):
 - Host passes node_feat TRANSPOSED (feature-major [256, N]) so the big
   matmul needs no on-chip input transpose, plus a one-hot segment
   matrix [64, N] so the per-node text gather becomes two tiny-K
   matmuls (uniform across cores).
 - All matmuls run as float32r (full-rate fp32, ~1e-4 rel err).
 - Feature-major epilogue: sigmoid on ACT, gate*t_node on DVE,
   +node on GPSIMD, then PE transpose-mode flips 128x128 blocks to
   node-major PSUM where LayerNorm runs (bn_stats + Newton rsqrt +
   ACT affine).
"""

import os
import sys

for _p in ("/opt/trn_rl_repo", "/root/.axon_site/_ro/trn_rl_repo"):
    if os.path.isdir(_p) and _p not in sys.path:
        sys.path.insert(0, _p)

from contextlib import ExitStack

import numpy as np

import concourse.bacc as bacc
import concourse.mybir as mybir
import concourse.tile as tile
from concourse.bass_utils import run_bass_kernel_spmd
from concourse.masks import make_identity

F32 = mybir.dt.float32
F32R = mybir.dt.float32r
AF = mybir.ActivationFunctionType
N_CORES = 8
D = 256          # node dim
TD = 768         # text dim
HD = 1024        # hidden dim
B = 64           # batch (segments)
CHUNK = 512      # nodes per inner chunk
LN_EPS = 1e-3


def _build(npc: int, apply_gb: bool):
    """Build the single SPMD program for `npc` nodes per core."""
    nch = npc // CHUNK
    nc = bacc.Bacc("TRN2", target_bir_lowering=False, debug=False,
                   num_devices=N_CORES)

    nodeT = nc.dram_tensor("nodeT", [D, npc], F32, kind="ExternalInput")
    onehot = nc.dram_tensor("onehot", [B, npc], mybir.dt.uint8, kind="ExternalInput")
    textT = nc.dram_tensor("textT", [TD, B], F32, kind="ExternalInput")
    w1 = nc.dram_tensor("w1", [TD, HD], F32, kind="ExternalInput")
    b1 = nc.dram_tensor("b1", [1, HD], F32, kind="ExternalInput")
    w2 = nc.dram_tensor("w2", [HD, D], F32, kind="ExternalInput")
    b2 = nc.dram_tensor("b2", [1, D], F32, kind="ExternalInput")
    wg1 = nc.dram_tensor("wg1", [D, D], F32, kind="ExternalInput")
    wg2 = nc.dram_tensor("wg2", [D, D], F32, kind="ExternalInput")
    bg = nc.dram_tensor("bg", [1, D], F32, kind="ExternalInput")
    gamma = nc.dram_tensor("gamma", [1, D], F32, kind="ExternalInput")
    beta = nc.dram_tensor("beta", [1, D], F32, kind="ExternalInput")
    onesd = nc.dram_tensor("onesd", [1, B], F32, kind="ExternalInput")
    out = nc.dram_tensor("out", [npc, D], F32, kind="ExternalOutput")

    with tile.TileContext(nc) as tc:
        with ExitStack() as ctx:
            consts = ctx.enter_context(tc.tile_pool(name="consts", bufs=1))

            # ---- constants / weights in SBUF ----
            wg1_sb = consts.tile([128, 2, D], F32R)
            nc.sync.dma_start(out=wg1_sb, in_=wg1.bitcast(F32R).rearrange("(c k) n -> k c n", c=2))
            b1_sb = consts.tile([1, HD], F32R)
            nc.sync.dma_start(out=b1_sb, in_=b1.bitcast(F32R)[:, :])
            b2_sb = consts.tile([1, D], F32R)
            nc.sync.dma_start(out=b2_sb, in_=b2.bitcast(F32R)[:, :])
            bg_sb = consts.tile([1, D], F32R)
            nc.sync.dma_start(out=bg_sb, in_=bg.bitcast(F32R)[:, :])
            ones64 = consts.tile([1, B], F32R)
            nc.sync.dma_start(out=ones64, in_=onesd.bitcast(F32R)[:, :])
            ident = consts.tile([128, 128], F32)
            make_identity(nc, ident)
            t_sb = consts.tile([B, D], F32R)    # text rows, node-dim
            u_sb = consts.tile([B, D], F32R)    # (t @ Wg2 + bg) rows

            def R(ap):
                return ap.bitcast(F32R)

            # ---- text MLP (one-time, tiny) ----
            with ExitStack() as mctx:
                mp = mctx.enter_context(tc.tile_pool(name="mlp", bufs=1))
                mps = mctx.enter_context(
                    tc.tile_pool(name="mlp_ps", bufs=1, space="PSUM"))
                tx_sb = mp.tile([128, 6, B], F32R)
                nc.sync.dma_start(out=tx_sb, in_=textT.bitcast(F32R).rearrange("(c k) m -> k c m", c=6))
                w1_sb = mp.tile([128, 6, HD], F32R)
                nc.sync.dma_start(out=w1_sb, in_=w1.bitcast(F32R).rearrange("(c k) n -> k c n", c=6))
                w2_sb = mp.tile([128, 8, D], F32R)
                nc.sync.dma_start(out=w2_sb, in_=w2.bitcast(F32R).rearrange("(c k) n -> k c n", c=8))
                wg2_sb = mp.tile([128, 2, D], F32R)
                nc.sync.dma_start(out=wg2_sb, in_=wg2.bitcast(F32R).rearrange("(c k) n -> k c n", c=2))
                ps_t1 = mps.tile([B, 2, 512], F32)
                for h in range(2):
                    for k in range(6):
                        nc.tensor.matmul(
                            ps_t1[:, h, :], R(tx_sb[:, k, :]),
                            R(w1_sb[:, k, h * 512:(h + 1) * 512]),
                            start=(k == 0), stop=False)
                    nc.tensor.matmul(
                        ps_t1[:, h, :], R(ones64),
                        R(b1_sb[:, h * 512:(h + 1) * 512]),
                        start=False, stop=True)
                t1_sb = mp.tile([B, 2, 512], F32)
                for h in range(2):
                    nc.scalar.activation(out=t1_sb[:, h, :], in_=ps_t1[:, h, :],
                                         func=AF.Relu)
                # transpose t1 -> t1T [1024, 64] as [128, 8, 64]
                t1T_sb = mp.tile([128, 8, B], F32R)
                ps_tr = mps.tile([128, B], F32)
                for j in range(8):
                    src = t1_sb[:, j // 4, (j % 4) * 128:(j % 4 + 1) * 128]
                    nc.tensor.matmul(ps_tr, src, ident[:B, :B],
                                     is_transpose=True, start=True, stop=True)
                    nc.vector.tensor_copy(out=t1T_sb[:, j, :], in_=ps_tr)
                ps_t = mps.tile([B, D], F32)
                for j in range(8):
                    nc.tensor.matmul(ps_t, R(t1T_sb[:, j, :]), R(w2_sb[:, j, :]),
                                     start=(j == 0), stop=False)
                nc.tensor.matmul(ps_t, R(ones64), R(b2_sb), start=False, stop=True)
                nc.vector.tensor_copy(out=t_sb, in_=ps_t)
                # transpose t -> tT [256, 64] as [128, 2, 64]
                tT_sb = mp.tile([128, 2, B], F32R)
                for c in range(2):
                    nc.tensor.matmul(ps_tr, t_sb[:, c * 128:(c + 1) * 128].bitcast(F32),
                                     ident[:B, :B],
                                     is_transpose=True, start=True, stop=True)
                    nc.vector.tensor_copy(out=tT_sb[:, c, :], in_=ps_tr)
                ps_u = mps.tile([B, D], F32)
                for c in range(2):
                    nc.tensor.matmul(ps_u, R(tT_sb[:, c, :]), R(wg2_sb[:, c, :]),
                                     start=(c == 0), stop=False)
                nc.tensor.matmul(ps_u, R(ones64), R(bg_sb), start=False, stop=True)
                nc.vector.tensor_copy(out=u_sb, in_=ps_u)

            # ---- main loop ----
            inp = ctx.enter_context(tc.tile_pool(name="inp", bufs=5))
            work = ctx.enter_context(tc.tile_pool(name="work", bufs=4))
            pz = ctx.enter_context(tc.tile_pool(name="pz", bufs=2, space="PSUM"))
            ptn = ctx.enter_context(tc.tile_pool(name="ptn", bufs=1, space="PSUM"))
            pe_ps = ctx.enter_context(tc.tile_pool(name="pe_ps", bufs=1, space="PSUM"))

            nodeTv = nodeT.bitcast(F32R).rearrange("(c k) n -> k c n", c=2)
            outv = out.rearrange("(ch j p) f -> ch p j f", p=128, j=4)
            outv2 = out.rearrange("(c2 j p) f -> c2 p j f", p=128, j=8)

            gb_sb = None
            if apply_gb:
                gb_sb = consts.tile([128, 2, D], F32)
                for name, src, slot in (("g", gamma, 0), ("b", beta, 1)):
                    import concourse.bass as bass
                    bcast = bass.AP(tensor=src.ap().tensor, offset=0,
                                    ap=[[0, 128], [1, D]])
                    nc.gpsimd.dma_start(out=gb_sb[:, slot, :], in_=bcast)

            dma_cache = {}

            def front_half(ch):
                """DMA-in + matmuls + sigmoid/mul/add for chunk ch.
                Returns the live enh tile for the back half."""
                # node: 2-chunk DMAs on the SP ring; onehot: 4-chunk DMAs
                # via SWDGE (gpsimd) so the two never share a DGE queue.
                if ch % 2 == 0:
                    n2 = inp.tile([128, 2, 2 * CHUNK], F32R, tag="node2")
                    hi = min((ch + 2) * CHUNK, npc)
                    nc.sync.dma_start(out=n2[:, :, :hi - ch * CHUNK],
                                      in_=nodeTv[:, :, ch * CHUNK:hi])
                    dma_cache["node"] = n2
                if ch % 4 == 0:
                    o4 = inp.tile([B, 4 * CHUNK], F32R, tag="oh4")
                    hi = min((ch + 4) * CHUNK, npc)
                    # SWDGE casts uint8 -> f32r during the transfer, so the
                    # one-hot matrix costs 1 byte/elem of HBM instead of 4
                    nc.gpsimd.dma_start(out=o4[:, :hi - ch * CHUNK],
                                        in_=onehot[:, ch * CHUNK:hi])
                    dma_cache["oh"] = o4
                node_sb = dma_cache["node"][:, :, (ch % 2) * CHUNK:
                                            (ch % 2 + 1) * CHUNK]
                oh_sb = dma_cache["oh"][:, (ch % 4) * CHUNK:(ch % 4 + 1) * CHUNK]

                ps_z = pz.tile([128, 2, CHUNK], F32, tag="ps_z")
                ps_tn = ptn.tile([128, 2, CHUNK], F32, tag="ps_tn")
                for c in range(2):
                    for k in range(2):
                        nc.tensor.matmul(
                            ps_z[:, c, :],
                            R(wg1_sb[:, k, c * 128:(c + 1) * 128]),
                            R(node_sb[:, k, :]),
                            start=(k == 0), stop=False)
                    nc.tensor.matmul(
                        ps_z[:, c, :], R(u_sb[:, c * 128:(c + 1) * 128]),
                        R(oh_sb), start=False, stop=True)
                    nc.tensor.matmul(
                        ps_tn[:, c, :], R(t_sb[:, c * 128:(c + 1) * 128]),
                        R(oh_sb), start=True, stop=True)

                gate_sb = work.tile([128, 2, CHUNK], F32, tag="gate")
                gt_sb = work.tile([128, 2, CHUNK], F32, tag="gt")
                enh_sb = work.tile([128, 2, CHUNK], F32, tag="enh")
                # sigmoid/mul stay per-PSUM-bank (ops must not cross a
                # bank); the SBUF-only add fuses both banks into one
                # GPSIMD op to amortize its dispatch cost.
                for c in range(2):
                    nc.scalar.activation(out=gate_sb[:, c, :],
                                         in_=ps_z[:, c, :], func=AF.Sigmoid)
                    nc.vector.tensor_mul(out=gt_sb[:, c, :],
                                         in0=gate_sb[:, c, :],
                                         in1=ps_tn[:, c, :])
                nc.gpsimd.tensor_add(out=enh_sb[:, :, :],
                                     in0=gt_sb[:, :, :],
                                     in1=node_sb[:, :, :].bitcast(F32))
                return enh_sb

            def back_half(ch, enh_sb):
                """Transpose + LayerNorm + store for chunk ch."""
                ps_e = pe_ps.tile([128, 2, CHUNK], F32, tag="ps_e")
                for j in range(4):
                    for c in range(2):
                        nc.tensor.matmul(
                            ps_e[:, j // 2, (j % 2) * 256 + c * 128:
                                 (j % 2) * 256 + (c + 1) * 128],
                            enh_sb[:, c, j * 128:(j + 1) * 128],
                            ident, is_transpose=True,
                            start=True, stop=True, skip_group_check=True)

                st_sb = work.tile([128, 2, 2, 6], F32, tag="st")
                mv_sb = work.tile([128, 2, 2, 2], F32, tag="mv")
                for b in range(2):
                    for g in range(2):
                        nc.vector.bn_stats(
                            out=st_sb[:, b, g, :],
                            in_=ps_e[:, b, g * 256:(g + 1) * 256])
                        nc.vector.bn_aggr(out=mv_sb[:, b, g, :],
                                          in_=st_sb[:, b, g:g + 1, :])
                # rstd = 1/sqrt(var+eps): recip-seeded Newton (1 iter; var~1.1)
                ve = work.tile([128, 2, 2, 1], F32, tag="ve")
                y = work.tile([128, 2, 2, 1], F32, tag="y")
                tmp = work.tile([128, 2, 2, 1], F32, tag="tmp")
                negms = work.tile([128, 2, 2, 1], F32, tag="negms")
                nc.vector.tensor_scalar_add(out=ve, in0=mv_sb[:, :, :, 1:2],
                                            scalar1=LN_EPS)
                nc.vector.reciprocal(out=y, in_=ve)
                nc.vector.tensor_scalar(out=y, in0=y, scalar1=0.5, scalar2=0.5,
                                        op0=mybir.AluOpType.mult,
                                        op1=mybir.AluOpType.add)
                for _ in range(2):
                    nc.vector.tensor_mul(out=tmp, in0=y, in1=y)
                    nc.vector.tensor_mul(out=tmp, in0=tmp, in1=ve)
                    nc.vector.tensor_scalar(out=tmp, in0=tmp, scalar1=-0.5,
                                            scalar2=1.5,
                                            op0=mybir.AluOpType.mult,
                                            op1=mybir.AluOpType.add)
                    nc.vector.tensor_mul(out=y, in0=y, in1=tmp)
                nc.vector.tensor_mul(out=negms, in0=mv_sb[:, :, :, 0:1], in1=y)
                nc.vector.tensor_scalar_mul(out=negms, in0=negms, scalar1=-1.0)

                # pair output tiles of two chunks into one 1MB DMA
                if ch % 2 == 0:
                    out2_sb = work.tile([128, 8, D], F32, tag="out2")
                    dma_cache["out2"] = out2_sb
                out_sb = dma_cache["out2"][:, (ch % 2) * 4:(ch % 2) * 4 + 4, :]
                for b in range(2):
                    for g in range(2):
                        j = 2 * b + g
                        nc.scalar.activation(
                            out=out_sb[:, j, :],
                            in_=ps_e[:, b, g * 256:(g + 1) * 256],
                            func=AF.Identity,
                            bias=negms[:, b, g, :], scale=y[:, b, g, :])
                if apply_gb:
                    for j in range(4):
                        nc.vector.tensor_mul(out=out_sb[:, j, :],
                                             in0=out_sb[:, j, :],
                                             in1=gb_sb[:, 0, :])
                        nc.vector.tensor_add(out=out_sb[:, j, :],
                                             in0=out_sb[:, j, :],
                                             in1=gb_sb[:, 1, :])
                if ch % 2 == 1:
                    nc.scalar.dma_start(out=outv2[ch // 2],
                                        in_=dma_cache["out2"])
                elif ch == nch - 1:
                    nc.scalar.dma_start(out=outv[ch],
                                        in_=dma_cache["out2"][:, 0:4, :])

            # one-chunk software pipeline: chunk i's front half is emitted
            # before chunk i-1's back half so PE/ACT/DVE streams always have
            # ready work ahead of the cross-engine dependency chain.
            prev_enh = None
            for ch in range(nch + 1):
                if ch < nch:
                    cur_enh = front_half(ch)
                else:
                    cur_enh = None
                if prev_enh is not None:
                    back_half(ch - 1, prev_enh)
                prev_enh = cur_enh

    nc.compile()
    return nc


_NC_CACHE = {}


def kernel(node_feat, text_feat, segment_ids, W1, b1, W2, b2, Wg, bg,
           ln_gamma, ln_beta):
    total, d = node_feat.shape
    npc = total // N_CORES
    assert npc % CHUNK == 0

    node_feat = np.asarray(node_feat, dtype=np.float32)
    nodeT = np.ascontiguousarray(node_feat.T)               # [256, total]
    textT = np.ascontiguousarray(np.asarray(text_feat, np.float32).T)
    seg = np.asarray(segment_ids)
    onehot = (seg[None, :] == np.arange(B, dtype=seg.dtype)[:, None]
              ).astype(np.uint8)                            # [64, total]

    apply_gb = not (np.all(np.asarray(ln_gamma) == 1.0)
                    and np.all(np.asarray(ln_beta) == 0.0))

    key = (npc, apply_gb)
    if key not in _NC_CACHE:
        _NC_CACHE[key] = _build(npc, apply_gb)
    nc = _NC_CACHE[key]

    shared = {
        "textT": textT,
        "w1": np.asarray(W1, np.float32),
        "b1": np.asarray(b1, np.float32).reshape(1, HD),
        "w2": np.asarray(W2, np.float32),
        "b2": np.asarray(b2, np.float32).reshape(1, D),
        "wg1": np.ascontiguousarray(np.asarray(Wg, np.float32)[:D]),
        "wg2": np.ascontiguousarray(np.asarray(Wg, np.float32)[D:]),
        "bg": np.asarray(bg, np.float32).reshape(1, D),
        "gamma": np.asarray(ln_gamma, np.float32).reshape(1, D),
        "beta": np.asarray(ln_beta, np.float32).reshape(1, D),
        "onesd": np.ones((1, B), np.float32),
    }
    in_maps = []
    for c in range(N_CORES):
        m = dict(shared)
        m["nodeT"] = np.ascontiguousarray(nodeT[:, c * npc:(c + 1) * npc])
        m["onehot"] = np.ascontiguousarray(onehot[:, c * npc:(c + 1) * npc])
        in_maps.append(m)

    res = run_bass_kernel_spmd(nc, in_maps, core_ids=list(range(N_CORES)))
    out = np.concatenate([res.results[c]["out"] for c in range(N_CORES)], axis=0)
    return out.astype(np.float32)


def bench_device(inputs, iters=6):
    """Time repeated on-device executions (8 cores, inputs device-resident).

    Returns median seconds per execution (max over cores, incl. PJRT
    dispatch overhead of ~1ms)."""
    import time

    import jax
    import jax.numpy as jnp
    from jax.experimental.shard_map import shard_map
    from jax.sharding import Mesh, PartitionSpec

    import concourse.bass2jax as b2j
    import concourse.mybir as mb

    node_feat = np.asarray(inputs["node_feat"], np.float32)
    total = node_feat.shape[0]
    npc = total // N_CORES
    seg = np.asarray(inputs["segment_ids"])
    nodeT = np.ascontiguousarray(node_feat.T)
    onehot = (seg[None, :] == np.arange(B, dtype=seg.dtype)[:, None]
              ).astype(np.uint8)
    key = (npc, False)
    if key not in _NC_CACHE:
        _NC_CACHE[key] = _build(npc, False)
    nc = _NC_CACHE[key]
    shared = {
        "textT": np.ascontiguousarray(np.asarray(inputs["text_feat"], np.float32).T),
        "w1": np.asarray(inputs["W1"], np.float32),
        "b1": np.asarray(inputs["b1"], np.float32).reshape(1, HD),
        "w2": np.asarray(inputs["W2"], np.float32),
        "b2": np.asarray(inputs["b2"], np.float32).reshape(1, D),
        "wg1": np.ascontiguousarray(np.asarray(inputs["Wg"], np.float32)[:D]),
        "wg2": np.ascontiguousarray(np.asarray(inputs["Wg"], np.float32)[D:]),
        "bg": np.asarray(inputs["bg"], np.float32).reshape(1, D),
        "gamma": np.asarray(inputs["ln_gamma"], np.float32).reshape(1, D),
        "beta": np.asarray(inputs["ln_beta"], np.float32).reshape(1, D),
        "onesd": np.ones((1, B), np.float32),
    }
    in_maps = []
    for c in range(N_CORES):
        m = dict(shared)
        m["nodeT"] = np.ascontiguousarray(nodeT[:, c * npc:(c + 1) * npc])
        m["onehot"] = np.ascontiguousarray(onehot[:, c * npc:(c + 1) * npc])
        in_maps.append(m)

    b2j.install_neuronx_cc_hook()
    partition_name = (nc.partition_id_tensor.name
                      if nc.partition_id_tensor else None)
    in_names, out_names, out_avals, zero_outs = [], [], [], []
    for alloc in nc.m.functions[0].allocations:
        if not isinstance(alloc, mb.MemoryLocationSet):
            continue
        name = alloc.memorylocations[0].name
        if alloc.kind == "ExternalInput":
            if name != partition_name:
                in_names.append(name)
        elif alloc.kind == "ExternalOutput":
            out_names.append(name)
            shape = tuple(alloc.tensor_shape)
            dtype = mb.dt.np(alloc.dtype)
            out_avals.append(jax.core.ShapedArray(shape, dtype))
            zero_outs.append(np.zeros(shape, dtype))
    n_params = len(in_names)
    n_outs = len(out_avals)
    in_names_all = list(in_names) + out_names
    if partition_name is not None:
        in_names_all.append(partition_name)
    donate = tuple(range(n_params, n_params + n_outs))

    def _body(*args):
        operands = list(args)
        if partition_name is not None:
            operands.append(b2j.partition_id_tensor())
        outs = b2j._bass_exec_p.bind(
            *operands, out_avals=tuple(out_avals), in_names=tuple(in_names_all),
            out_names=tuple(out_names), lowering_input_output_aliases=(),
            sim_require_finite=True, sim_require_nnan=True, nc=nc)
        return tuple(outs)

    devices = jax.devices()[:N_CORES]
    mesh = Mesh(np.asarray(devices), ("core",))
    sharded = jax.jit(
        shard_map(_body, mesh=mesh,
                  in_specs=(PartitionSpec("core"),) * (n_params + n_outs),
                  out_specs=(PartitionSpec("core"),) * n_outs,
                  check_rep=False),
        donate_argnums=donate, keep_unused=True)
    concat_in = [
        np.concatenate([np.asarray(in_maps[c][nm]) for c in range(N_CORES)], axis=0)
        for nm in in_names]
    sh = jax.sharding.NamedSharding(mesh, PartitionSpec("core"))
    in_dev = [jax.device_put(a, sh) for a in concat_in]
    times = []
    for it in range(iters):
        zs = [jax.device_put(
            np.zeros((N_CORES * z.shape[0], *z.shape[1:]), z.dtype), sh)
            for z in zero_outs]
        jax.block_until_ready(zs)
        t0 = time.perf_counter()
        outs = sharded(*in_dev, *zs)
        jax.block_until_ready(outs)
        times.append(time.perf_counter() - t0)
    times.sort()
    return times[len(times) // 2], times


def run_traced(inputs):
    """Re-run with NTFF tracing; returns max-core exec time in ns (or None)."""
    global _LAST_TRACE
    import kernel as K  # ensure cache shared

    node_feat = np.asarray(inputs["node_feat"], np.float32)
    total = node_feat.shape[0]
    npc = total // N_CORES
    seg = np.asarray(inputs["segment_ids"])
    nodeT = np.ascontiguousarray(node_feat.T)
    onehot = (seg[None, :] == np.arange(B, dtype=seg.dtype)[:, None]
              ).astype(np.uint8)
    apply_gb = not (np.all(np.asarray(inputs["ln_gamma"]) == 1.0)
                    and np.all(np.asarray(inputs["ln_beta"]) == 0.0))
    key = (npc, apply_gb)
    if key not in _NC_CACHE:
        _NC_CACHE[key] = _build(npc, apply_gb)
    nc = _NC_CACHE[key]
    shared = {
        "textT": np.ascontiguousarray(np.asarray(inputs["text_feat"], np.float32).T),
        "w1": np.asarray(inputs["W1"], np.float32),
        "b1": np.asarray(inputs["b1"], np.float32).reshape(1, HD),
        "w2": np.asarray(inputs["W2"], np.float32),
        "b2": np.asarray(inputs["b2"], np.float32).reshape(1, D),
        "wg1": np.ascontiguousarray(np.asarray(inputs["Wg"], np.float32)[:D]),
        "wg2": np.ascontiguousarray(np.asarray(inputs["Wg"], np.float32)[D:]),
        "bg": np.asarray(inputs["bg"], np.float32).reshape(1, D),
        "gamma": np.asarray(inputs["ln_gamma"], np.float32).reshape(1, D),
        "beta": np.asarray(inputs["ln_beta"], np.float32).reshape(1, D),
        "onesd": np.ones((1, B), np.float32),
    }
    in_maps = []
    for c in range(N_CORES):
        m = dict(shared)
        m["nodeT"] = np.ascontiguousarray(nodeT[:, c * npc:(c + 1) * npc])
        m["onehot"] = np.ascontiguousarray(onehot[:, c * npc:(c + 1) * npc])
        in_maps.append(m)
    res = run_bass_kernel_spmd(nc, in_maps, core_ids=list(range(N_CORES)),
                               trace=True)
    _LAST_TRACE = res
    return res.exec_time_ns



# revision 2
# speedup vs baseline: 460.2768x; 460.2768x over previous
"""Trainium2 Bass kernel for nn_MiddleFusionModule.

out = LayerNorm(node + sigmoid(node@Wg1 + (t@Wg2+bg)[seg]) * t[seg]),
t = relu(text@W1+b1)@W2+b2, over 131072 nodes sharded data-parallel
across 8 cores (16384 nodes each; LayerNorm is per-node so there is
no cross-shard communication).

Dataflow per 512-node chunk (one SPMD program on every core):
 - host sends node_feat transposed to feature-major bf16 [256, N] (half
   the HBM traffic of f32) plus a uint8 one-hot segment matrix [64, N];
   SWDGE casts the one-hot to f32r in-flight.
 - z = Wg1^T@node (bf16 matmuls) + u[seg] via one-hot matmul (f32r,
   full rate at >=256 moving columns); tn = t[seg] likewise.
 - ACT sigmoid -> DVE gt = gate*tn -> DVE enh = gt + node (bf16).
 - PE transposes enh into a single-bank bf16 PSUM tile (node-major);
   LN stats come from skinny PE matmuls against a ones column (sum,
   and sum of a DVE-squared copy), so DVE never runs bn_stats.
 - rstd = 1/sqrt(var+eps) via DVE reciprocal + minimax-seeded Newton
   (one iteration) on the Pool engine; final per-group affine on ACT;
   per-chunk 512KB output DMAs on the SP queue.
 - PSUM: ps_z + ps_tn (f32, 2 banks each) + ps_e (bf16, 1 bank x3
   bufs) + ps_s (1 bank) = 8 banks, so chunks pipeline 4 deep
   (lookahead=4) with no bank stalls.

Measured on trn2: ~187 us/pass per core-group (vs 389 us for the f32
v1), rel err 3.9e-3 (gate 2e-2). Engine busy (sim): DMA 95us,
ACT 94us, DVE 93us, PE 82us.
"""

import os
import sys

for _p in ("/opt/trn_rl_repo", "/root/.axon_site/_ro/trn_rl_repo"):
    if os.path.isdir(_p) and _p not in sys.path:
        sys.path.insert(0, _p)

from contextlib import ExitStack

import numpy as np

import concourse.bacc as bacc
import concourse.mybir as mybir
import concourse.tile as tile
from concourse.bass_utils import run_bass_kernel_spmd
from concourse.masks import make_identity

F32 = mybir.dt.float32
F32R = mybir.dt.float32r
BF16 = mybir.dt.bfloat16
F8 = mybir.dt.float8e4
U8 = mybir.dt.uint8
WG1_SCALE = 32.0  # wg1/u pre-scale so fp8e4m3 stays in its normal range
AF = mybir.ActivationFunctionType
N_CORES = 8
D = 256          # node dim
TD = 768         # text dim
HD = 1024        # hidden dim
B = 64           # batch (segments)
CHUNK = 512      # nodes per inner chunk
LN_EPS = 1e-3


DEFAULT_TUNE = dict(pz_bufs=1, ptn_bufs=1, pe_bufs=3, lookahead=4,
                    enh_engine="dve", enh_split=2, z_fp8=False,
                    stats="skinny", chunk=512, n_affine_dve=0,
                    newton_eng="pool", newton="fast1", interleave_c=True,
                    out_batch=512, out_q="sync")


def _build(npc: int, apply_gb: bool, reps: int = 1, tune: dict | None = None):
    """Build the single SPMD program for `npc` nodes per core.

    reps>1 re-runs the whole main loop (same I/O) for differential
    HW timing; output is identical.
    """
    tn_cfg = dict(DEFAULT_TUNE)
    if tune:
        tn_cfg.update(tune)
    CH = tn_cfg["chunk"]          # nodes per inner chunk
    NG = CH // 128                # node groups per chunk
    NB = tn_cfg.get("in_batch", 2048) // CH   # chunks per node/oh DMA batch
    OB = tn_cfg.get("out_batch", 1024) // CH  # chunks per output DMA batch
    nch = npc // CH
    nc = bacc.Bacc("TRN2", target_bir_lowering=False, debug=False,
                   num_devices=N_CORES)

    nodeT = nc.dram_tensor("nodeT", [D, npc], BF16, kind="ExternalInput")
    nodeF8 = (nc.dram_tensor("nodeF8", [D, npc], F8, kind="ExternalInput")
              if tn_cfg["z_fp8"] else None)
    onehot = nc.dram_tensor("onehot", [B, npc], U8, kind="ExternalInput")
    textT = nc.dram_tensor("textT", [TD, B], F32, kind="ExternalInput")
    w1 = nc.dram_tensor("w1", [TD, HD], F32, kind="ExternalInput")
    b1 = nc.dram_tensor("b1", [1, HD], F32, kind="ExternalInput")
    w2 = nc.dram_tensor("w2", [HD, D], F32, kind="ExternalInput")
    b2 = nc.dram_tensor("b2", [1, D], F32, kind="ExternalInput")
    wg1 = (nc.dram_tensor("wg1", [D, D], F8, kind="ExternalInput")
           if tn_cfg["z_fp8"] else None)
    wg1b = (None if tn_cfg["z_fp8"]
            else nc.dram_tensor("wg1b", [D, D], BF16, kind="ExternalInput"))
    wg2 = nc.dram_tensor("wg2", [D, D], F32, kind="ExternalInput")
    bg = nc.dram_tensor("bg", [1, D], F32, kind="ExternalInput")
    gamma = nc.dram_tensor("gamma", [1, D], F32, kind="ExternalInput")
    beta = nc.dram_tensor("beta", [1, D], F32, kind="ExternalInput")
    onesd = nc.dram_tensor("onesd", [1, B], F32, kind="ExternalInput")
    out = nc.dram_tensor("out", [npc, D], F32, kind="ExternalOutput")

    with tile.TileContext(nc) as tc:
        with ExitStack() as ctx:
            consts = ctx.enter_context(tc.tile_pool(name="consts", bufs=1))

            # ---- constants / weights in SBUF ----
            if tn_cfg["z_fp8"]:
                # wg1 as fp8 in DoubleRow k-tile layout: [128, ktile, 256]
                wg1_sb = consts.tile([128, 2, D], F8)
                nc.sync.dma_start(out=wg1_sb,
                                  in_=wg1.rearrange("(t k) n -> k t n", t=2))
            else:
                wg1b_sb = consts.tile([128, 2, D], BF16)
                nc.sync.dma_start(out=wg1b_sb,
                                  in_=wg1b.rearrange("(c k) n -> k c n", c=2))
            b1_sb = consts.tile([1, HD], F32R)
            nc.sync.dma_start(out=b1_sb, in_=b1.bitcast(F32R)[:, :])
            b2_sb = consts.tile([1, D], F32R)
            nc.sync.dma_start(out=b2_sb, in_=b2.bitcast(F32R)[:, :])
            bg_sb = consts.tile([1, D], F32R)
            nc.sync.dma_start(out=bg_sb, in_=bg.bitcast(F32R)[:, :])
            ones64 = consts.tile([1, B], F32R)
            nc.sync.dma_start(out=ones64, in_=onesd.bitcast(F32R)[:, :])
            ident = consts.tile([128, 128], F32)
            make_identity(nc, ident)
            identB = consts.tile([128, 128], BF16)
            make_identity(nc, identB)
            onecol = consts.tile([128, 1], BF16)
            nc.gpsimd.memset(onecol, 1.0)
            t_sb = consts.tile([B, D], F32R)    # text rows, node-dim
            u_sb = consts.tile([B, D], F32R)    # (t @ Wg2 + bg) rows

            def R(ap):
                return ap.bitcast(F32R)

            # ---- text MLP (one-time, tiny, f32r) ----
            with ExitStack() as mctx:
                mp = mctx.enter_context(tc.tile_pool(name="mlp", bufs=1))
                mps = mctx.enter_context(
                    tc.tile_pool(name="mlp_ps", bufs=1, space="PSUM"))
                tx_sb = mp.tile([128, 6, B], F32R)
                nc.sync.dma_start(out=tx_sb, in_=textT.bitcast(F32R).rearrange("(c k) m -> k c m", c=6))
                w1_sb = mp.tile([128, 6, HD], F32R)
                nc.sync.dma_start(out=w1_sb, in_=w1.bitcast(F32R).rearrange("(c k) n -> k c n", c=6))
                w2_sb = mp.tile([128, 8, D], F32R)
                nc.sync.dma_start(out=w2_sb, in_=w2.bitcast(F32R).rearrange("(c k) n -> k c n", c=8))
                wg2_sb = mp.tile([128, 2, D], F32R)
                nc.sync.dma_start(out=wg2_sb, in_=wg2.bitcast(F32R).rearrange("(c k) n -> k c n", c=2))
                ps_t1 = mps.tile([B, 2, 512], F32)
                for h in range(2):
                    for k in range(6):
                        nc.tensor.matmul(
                            ps_t1[:, h, :], R(tx_sb[:, k, :]),
                            R(w1_sb[:, k, h * 512:(h + 1) * 512]),
                            start=(k == 0), stop=False)
                    nc.tensor.matmul(
                        ps_t1[:, h, :], R(ones64),
                        R(b1_sb[:, h * 512:(h + 1) * 512]),
                        start=False, stop=True)
                t1_sb = mp.tile([B, 2, 512], F32)
                for h in range(2):
                    nc.scalar.activation(out=t1_sb[:, h, :], in_=ps_t1[:, h, :],
                                         func=AF.Relu)
                # transpose t1 -> t1T [1024, 64] as [128, 8, 64]
                t1T_sb = mp.tile([128, 8, B], F32R)
                ps_tr = mps.tile([128, B], F32)
                for j in range(8):
                    src = t1_sb[:, j // 4, (j % 4) * 128:(j % 4 + 1) * 128]
                    nc.tensor.matmul(ps_tr, src, ident[:B, :B],
                                     is_transpose=True, start=True, stop=True)
                    nc.vector.tensor_copy(out=t1T_sb[:, j, :], in_=ps_tr)
                ps_t = mps.tile([B, D], F32)
                for j in range(8):
                    nc.tensor.matmul(ps_t, R(t1T_sb[:, j, :]), R(w2_sb[:, j, :]),
                                     start=(j == 0), stop=False)
                nc.tensor.matmul(ps_t, R(ones64), R(b2_sb), start=False, stop=True)
                t_f32 = mp.tile([B, D], F32)
                nc.vector.tensor_copy(out=t_f32, in_=ps_t)
                nc.vector.tensor_copy(out=t_sb, in_=ps_t)
                # transpose t -> tT [256, 64] as [128, 2, 64]
                tT_sb = mp.tile([128, 2, B], F32R)
                for c in range(2):
                    nc.tensor.matmul(ps_tr, t_f32[:, c * 128:(c + 1) * 128],
                                     ident[:B, :B],
                                     is_transpose=True, start=True, stop=True)
                    nc.vector.tensor_copy(out=tT_sb[:, c, :], in_=ps_tr)
                ps_u = mps.tile([B, D], F32)
                for c in range(2):
                    nc.tensor.matmul(ps_u, R(tT_sb[:, c, :]), R(wg2_sb[:, c, :]),
                                     start=(c == 0), stop=False)
                nc.tensor.matmul(ps_u, R(ones64), R(bg_sb), start=False, stop=True)
                # u is pre-scaled by WG1_SCALE to match the scaled fp8 wg1;
                # the sigmoid rescales by 1/WG1_SCALE.
                nc.vector.tensor_scalar_mul(
                    out=u_sb, in0=ps_u,
                    scalar1=WG1_SCALE if tn_cfg["z_fp8"] else 1.0)

            # ---- main loop ----
            inp = ctx.enter_context(tc.tile_pool(name="inp", bufs=3))
            ohp = ctx.enter_context(tc.tile_pool(name="ohp", bufs=2))
            work = ctx.enter_context(
                tc.tile_pool(name="work", bufs=3 + tn_cfg["lookahead"]))
            pz = ctx.enter_context(
                tc.tile_pool(name="pz", bufs=tn_cfg["pz_bufs"], space="PSUM"))
            ptn = ctx.enter_context(
                tc.tile_pool(name="ptn", bufs=tn_cfg["ptn_bufs"], space="PSUM"))
            pe_ps = ctx.enter_context(
                tc.tile_pool(name="pe_ps", bufs=tn_cfg["pe_bufs"], space="PSUM"))
            pss = None
            if tn_cfg["stats"] in ("ttr", "skinny"):
                pss = ctx.enter_context(
                    tc.tile_pool(name="pss", bufs=1, space="PSUM"))

            nodeTv = nodeT.rearrange("(c k) n -> k c n", c=2)
            nodeF8v = (nodeF8.rearrange("(c k) n -> k c n", c=2)
                       if tn_cfg["z_fp8"] else None)
            outv = out.rearrange("(ch j p) f -> ch p j f", p=128, j=NG)
            outv2 = out.rearrange("(c2 j p) f -> c2 p j f", p=128, j=OB * NG)

            gb_sb = None
            if apply_gb:
                gb_sb = consts.tile([128, 2, D], F32)
                for name, src, slot in (("g", gamma, 0), ("b", beta, 1)):
                    import concourse.bass as bass
                    bcast = bass.AP(tensor=src.ap().tensor, offset=0,
                                    ap=[[0, 128], [1, D]])
                    nc.gpsimd.dma_start(out=gb_sb[:, slot, :], in_=bcast)

            dma_cache = {}

            def front_half(ch):
                """DMA-in + matmuls + sigmoid/mul/add for chunk ch.
                Returns the live enh tile for the back half."""
                # node: 2048-node DMAs on the SP ring; onehot likewise via
                # SWDGE (gpsimd) so the two never share a DGE queue.
                if ch % NB == 0:
                    n2 = inp.tile([128, 2, NB * CH], BF16, tag="node2")
                    hi = min((ch + NB) * CH, npc)
                    nc.sync.dma_start(out=n2[:, :, :hi - ch * CH],
                                      in_=nodeTv[:, :, ch * CH:hi])
                    dma_cache["node"] = n2
                    if tn_cfg["z_fp8"]:
                        f2 = inp.tile([128, 2, NB * CH], F8, tag="node8")
                        nc.sync.dma_start(out=f2[:, :, :hi - ch * CH],
                                          in_=nodeF8v[:, :, ch * CH:hi])
                        dma_cache["node8"] = f2
                    o4 = ohp.tile([B, NB * CH], F32R, tag="oh4")
                    # SWDGE casts uint8 -> f32r during the transfer, so the
                    # one-hot matrix costs 1 byte/elem of HBM instead of 4
                    nc.gpsimd.dma_start(out=o4[:, :hi - ch * CH],
                                        in_=onehot[:, ch * CH:hi])
                    dma_cache["oh"] = o4
                node_sb = dma_cache["node"][:, :, (ch % NB) * CH:
                                            (ch % NB + 1) * CH]
                node8_sb = (dma_cache["node8"][:, :, (ch % NB) * CH:
                                               (ch % NB + 1) * CH]
                            if tn_cfg["z_fp8"] else None)
                oh_sb = dma_cache["oh"][:, (ch % NB) * CH:(ch % NB + 1) * CH]

                ps_z = pz.tile([128, 2, CH], F32, tag="ps_z")
                ps_tn = ptn.tile([128, 2, CH], F32, tag="ps_tn")
                gate_sb = work.tile([128, 2, CH], F32, tag="gate")
                gt_sb = work.tile([128, 2, CH], BF16, tag="gt")
                enh_sb = work.tile([128, 2, CH], BF16, tag="enh")
                eng = nc.vector if tn_cfg["enh_engine"] == "dve" else nc.gpsimd
                sig_scale = 1.0 / WG1_SCALE if tn_cfg["z_fp8"] else 1.0

                def z_mm(c):
                    # the z accumulation closes its matmul group before the
                    # f32r u-gather piles on (start=False) so each group
                    # keeps a single operand dtype
                    if tn_cfg["z_fp8"]:
                        # z main term: one fp8 DoubleRow matmul covers both
                        # 128-deep k-tiles at 0.5 cycles/row
                        nc.tensor.matmul(
                            ps_z[:, c, :],
                            wg1_sb[:, :, c * 128:(c + 1) * 128],
                            node8_sb,
                            perf_mode=mybir.MatmulPerfMode.DoubleRow,
                            start=True, stop=True, skip_group_check=True)
                    else:
                        for k in range(2):
                            nc.tensor.matmul(
                                ps_z[:, c, :],
                                wg1b_sb[:, k, c * 128:(c + 1) * 128],
                                node_sb[:, k, :],
                                start=(k == 0), stop=(k == 1),
                                skip_group_check=True)
                    nc.tensor.matmul(
                        ps_z[:, c, :], R(u_sb[:, c * 128:(c + 1) * 128]),
                        R(oh_sb), start=False, stop=True,
                        skip_group_check=True)
                    nc.tensor.matmul(
                        ps_tn[:, c, :], R(t_sb[:, c * 128:(c + 1) * 128]),
                        R(oh_sb), start=True, stop=True)

                def sig_gt(c):
                    nc.scalar.activation(out=gate_sb[:, c, :],
                                         in_=ps_z[:, c, :], func=AF.Sigmoid,
                                         scale=sig_scale)
                    nc.vector.tensor_mul(out=gt_sb[:, c, :],
                                         in0=gate_sb[:, c, :],
                                         in1=ps_tn[:, c, :])

                if tn_cfg["interleave_c"]:
                    for c in range(2):
                        z_mm(c)
                        sig_gt(c)
                else:
                    for c in range(2):
                        z_mm(c)
                    for c in range(2):
                        sig_gt(c)
                if tn_cfg["enh_split"] == 1:
                    eng.tensor_add(out=enh_sb[:, :, :], in0=gt_sb[:, :, :],
                                   in1=node_sb[:, :, :])
                else:
                    for c in range(2):
                        eng.tensor_add(out=enh_sb[:, c, :],
                                       in0=gt_sb[:, c, :],
                                       in1=node_sb[:, c, :])
                return enh_sb

            def back_half(ch, enh_sb):
                """Transpose + LayerNorm + store for chunk ch."""
                ps_e = pe_ps.tile([128, NG, 256], BF16, tag="ps_e")
                for j in range(NG):
                    for c in range(2):
                        nc.tensor.matmul(
                            ps_e[:, j, c * 128:(c + 1) * 128],
                            enh_sb[:, c, j * 128:(j + 1) * 128],
                            identB, is_transpose=True,
                            start=True, stop=True, skip_group_check=True)

                y = work.tile([128, NG, 1], F32, tag="y")
                tmp = work.tile([128, NG, 1], F32, tag="tmp")
                negms = work.tile([128, NG, 1], F32, tag="negms")
                ve = work.tile([128, NG, 1], F32, tag="ve")
                if tn_cfg["stats"] == "ttr":
                    # mean via skinny PE matmuls on the feature-major enh
                    # (runs concurrently with the transposes); meansq via a
                    # fused DVE multiply-reduce on the transposed PSUM tile.
                    ps_s = pss.tile([128, NG, 1], F32, tag="ps_s")
                    for j in range(NG):
                        for c in range(2):
                            nc.tensor.matmul(
                                ps_s[:, j, :],
                                enh_sb[:, c, j * 128:(j + 1) * 128],
                                onecol, start=(c == 0), stop=(c == 1),
                                skip_group_check=True)
                    mu = work.tile([128, NG, 1], F32, tag="mu")
                    q = work.tile([128, NG, 1], F32, tag="q")
                    sqd = work.tile([128, NG, 256], BF16, tag="sqd")
                    for j in range(NG):
                        nc.vector.tensor_tensor_reduce(
                            out=sqd[:, j, :],
                            in0=ps_e[:, j, :],
                            in1=ps_e[:, j, :],
                            scale=1.0 / D, scalar=LN_EPS,
                            op0=mybir.AluOpType.mult,
                            op1=mybir.AluOpType.add,
                            accum_out=q[:, j, :])
                    nc.vector.tensor_scalar_mul(out=mu, in0=ps_s,
                                                scalar1=1.0 / D)
                    mu2 = work.tile([128, NG, 1], F32, tag="mu2")
                    nc.gpsimd.tensor_mul(out=mu2, in0=mu, in1=mu)
                    # ve = eps + E[x^2] - mu^2  (eps folded into the reduce)
                    nc.gpsimd.tensor_sub(out=ve, in0=q, in1=mu2)
                    mv0 = mu
                elif tn_cfg["stats"] == "skinny":
                    # both stats via skinny PE matmuls against a ones column:
                    # sum from enh, sumsq from a DVE-squared copy; all in
                    # feature-major so they overlap the transposes.
                    sq_sb = work.tile([128, 2, CH], BF16, tag="sq")
                    for c in range(2):
                        nc.vector.tensor_mul(out=sq_sb[:, c, :],
                                             in0=enh_sb[:, c, :],
                                             in1=enh_sb[:, c, :])
                    ps_s = pss.tile([128, NG, 2], F32, tag="ps_s")
                    for j in range(NG):
                        for c in range(2):
                            nc.tensor.matmul(
                                ps_s[:, j, 0:1],
                                enh_sb[:, c, j * 128:(j + 1) * 128],
                                onecol, start=(c == 0), stop=(c == 1),
                                skip_group_check=True)
                        for c in range(2):
                            nc.tensor.matmul(
                                ps_s[:, j, 1:2],
                                sq_sb[:, c, j * 128:(j + 1) * 128],
                                onecol, start=(c == 0), stop=(c == 1),
                                skip_group_check=True)
                    ms = work.tile([128, NG, 2], F32, tag="ms")
                    nc.vector.tensor_scalar_mul(out=ms, in0=ps_s,
                                                scalar1=1.0 / D)
                    mu2 = work.tile([128, NG, 1], F32, tag="mu2")
                    nc.gpsimd.tensor_mul(out=mu2, in0=ms[:, :, 0:1],
                                         in1=ms[:, :, 0:1])
                    nc.gpsimd.tensor_scalar(out=mu2, in0=mu2, scalar1=1.0,
                                            scalar2=-LN_EPS,
                                            op0=mybir.AluOpType.mult,
                                            op1=mybir.AluOpType.add)
                    # ve = E[x^2] - (mu^2 - eps)
                    nc.gpsimd.tensor_sub(out=ve, in0=ms[:, :, 1:2], in1=mu2)
                    mv0 = ms[:, :, 0:1]
                else:
                    st_sb = work.tile([128, NG, 6], F32, tag="st")
                    mv_sb = work.tile([128, NG, 2], F32, tag="mv")
                    for j in range(NG):
                        nc.vector.bn_stats(out=st_sb[:, j, :],
                                           in_=ps_e[:, j, :])
                    for j in range(NG):
                        nc.vector.bn_aggr(out=mv_sb[:, j, :],
                                          in_=st_sb[:, j:j + 1, :])
                    nc.gpsimd.tensor_scalar_add(out=ve, in0=mv_sb[:, :, 1:2],
                                                scalar1=LN_EPS)
                    mv0 = mv_sb[:, :, 0:1]
                # rstd = 1/sqrt(var+eps): recip-seeded Newton (2 iters;
                # var~1.1-1.9). The scalar tail runs on the Pool engine
                # (SBUF-only tiles) to keep DVE free; only the reciprocal
                # has no Pool equivalent, so it stays on DVE.
                ne = (nc.gpsimd if tn_cfg["newton_eng"] == "pool"
                      else nc.vector)
                nc.vector.reciprocal(out=y, in_=ve)
                if tn_cfg["newton"] == "fast1":
                    # minimax linear seed for sqrt(r) on r in [0.3, 1.0]
                    # (seed err <3.3% -> one Newton iter lands ~1e-3)
                    seed_a, seed_b, iters = 0.646, 0.371, 1
                else:
                    seed_a, seed_b, iters = 0.5, 0.5, 2
                ne.tensor_scalar(out=y, in0=y, scalar1=seed_a, scalar2=seed_b,
                                 op0=mybir.AluOpType.mult,
                                 op1=mybir.AluOpType.add)
                for _ in range(iters):
                    ne.tensor_mul(out=tmp, in0=y, in1=y)
                    ne.tensor_mul(out=tmp, in0=tmp, in1=ve)
                    ne.tensor_scalar(out=tmp, in0=tmp, scalar1=-0.5,
                                     scalar2=1.5,
                                     op0=mybir.AluOpType.mult,
                                     op1=mybir.AluOpType.add)
                    ne.tensor_mul(out=y, in0=y, in1=tmp)
                ne.tensor_mul(out=negms, in0=mv0, in1=y)
                ne.tensor_scalar_mul(out=negms, in0=negms, scalar1=-1.0)

                # pair output tiles of OB chunks into one out DMA
                if ch % OB == 0:
                    out2_sb = work.tile([128, OB * NG, D], F32, tag="out2")
                    dma_cache["out2"] = out2_sb
                out_sb = dma_cache["out2"][:, (ch % OB) * NG:
                                           (ch % OB + 1) * NG, :]
                for j in range(NG):
                    if j < tn_cfg["n_affine_dve"]:
                        # (x - mu) * rstd as a two-scalar DVE op
                        nc.vector.tensor_scalar(
                            out=out_sb[:, j, :], in0=ps_e[:, j, :],
                            scalar1=mv0[:, j, :], scalar2=y[:, j, :],
                            op0=mybir.AluOpType.subtract,
                            op1=mybir.AluOpType.mult)
                    else:
                        nc.scalar.activation(
                            out=out_sb[:, j, :],
                            in_=ps_e[:, j, :],
                            func=AF.Identity,
                            bias=negms[:, j, :], scale=y[:, j, :])
                if apply_gb:
                    for j in range(NG):
                        nc.vector.tensor_mul(out=out_sb[:, j, :],
                                             in0=out_sb[:, j, :],
                                             in1=gb_sb[:, 0, :])
                        nc.vector.tensor_add(out=out_sb[:, j, :],
                                             in0=out_sb[:, j, :],
                                             in1=gb_sb[:, 1, :])
                oq = {"scalar": nc.scalar, "pool": nc.gpsimd,
                      "sync": nc.sync, "vector": nc.vector}[
                          tn_cfg.get("out_q", "scalar")]
                if ch % OB == OB - 1:
                    oq.dma_start(out=outv2[ch // OB],
                                 in_=dma_cache["out2"])
                elif ch == nch - 1:
                    oq.dma_start(
                        out=outv[ch], in_=dma_cache["out2"][:, 0:NG, :])

            # software pipeline: chunk i+lookahead's front half is emitted
            # before chunk i's back half so PE/ACT/DVE streams always have
            # ready work ahead of the cross-engine dependency chain.
            LA = tn_cfg["lookahead"]
            total_ch = reps * nch
            pending = []
            for idx in range(total_ch + LA):
                if idx < total_ch:
                    pending.append((idx % nch, front_half(idx % nch)))
                if idx >= LA:
                    bch, benh = pending.pop(0)
                    back_half(bch, benh)

    nc.compile()
    return nc


_NC_CACHE = {}


def _get_nc(npc, apply_gb, reps=1):
    key = (npc, apply_gb, reps)
    if key not in _NC_CACHE:
        _NC_CACHE[key] = _build(npc, apply_gb, reps)
    return _NC_CACHE[key]


def _to_bf16(a):
    import ml_dtypes
    return np.asarray(a, np.float32).astype(ml_dtypes.bfloat16)


def _prepare_in_maps(node_feat, text_feat, segment_ids, W1, b1, W2, b2, Wg,
                     bg, ln_gamma, ln_beta, npc, z_fp8=None):
    if z_fp8 is None:
        z_fp8 = DEFAULT_TUNE["z_fp8"]
    node_feat = np.asarray(node_feat, dtype=np.float32)
    nodeT = _to_bf16(np.ascontiguousarray(node_feat.T))     # [256, total] bf16
    textT = np.ascontiguousarray(np.asarray(text_feat, np.float32).T)
    seg = np.asarray(segment_ids)
    onehot = (seg[None, :] == np.arange(B, dtype=seg.dtype)[:, None]
              ).astype(np.uint8)                            # [64, total]
    shared = {
        "textT": textT,
        "w1": np.asarray(W1, np.float32),
        "b1": np.asarray(b1, np.float32).reshape(1, HD),
        "w2": np.asarray(W2, np.float32),
        "b2": np.asarray(b2, np.float32).reshape(1, D),
        "wg2": np.ascontiguousarray(np.asarray(Wg, np.float32)[D:]),
        "bg": np.asarray(bg, np.float32).reshape(1, D),
        "gamma": np.asarray(ln_gamma, np.float32).reshape(1, D),
        "beta": np.asarray(ln_beta, np.float32).reshape(1, D),
        "onesd": np.ones((1, B), np.float32),
    }
    nodeF8 = None
    if z_fp8:
        shared["wg1"] = (np.asarray(Wg, np.float32)[:D] * WG1_SCALE).astype(
            mybir.dt.np(F8))
        nodeF8 = np.ascontiguousarray(node_feat.T).astype(mybir.dt.np(F8))
    else:
        shared["wg1b"] = _to_bf16(np.asarray(Wg, np.float32)[:D])
    in_maps = []
    for c in range(N_CORES):
        m = dict(shared)
        m["nodeT"] = np.ascontiguousarray(nodeT[:, c * npc:(c + 1) * npc])
        if nodeF8 is not None:
            m["nodeF8"] = np.ascontiguousarray(
                nodeF8[:, c * npc:(c + 1) * npc])
        m["onehot"] = np.ascontiguousarray(onehot[:, c * npc:(c + 1) * npc])
        in_maps.append(m)
    return in_maps


def kernel(node_feat, text_feat, segment_ids, W1, b1, W2, b2, Wg, bg,
           ln_gamma, ln_beta):
    total, d = node_feat.shape
    npc = total // N_CORES
    assert npc % CHUNK == 0

    apply_gb = not (np.all(np.asarray(ln_gamma) == 1.0)
                    and np.all(np.asarray(ln_beta) == 0.0))
    nc = _get_nc(npc, apply_gb)
    in_maps = _prepare_in_maps(node_feat, text_feat, segment_ids, W1, b1,
                               W2, b2, Wg, bg, ln_gamma, ln_beta, npc)
    res = run_bass_kernel_spmd(nc, in_maps, core_ids=list(range(N_CORES)))
    out = np.concatenate([res.results[c]["out"] for c in range(N_CORES)], axis=0)
    return out.astype(np.float32)
